# revision 30
# baseline (speedup 1.0000x reference)
"""Bilateral filter (3x3, sigma=0.8) Trainium2 Bass kernel — v7.

Sharding: fully data-parallel over the fused batch B*V = 8 -> one
(C=3,H=512,W=512) image per NeuronCore, 8 cores.

Per-core layout: H=512 rows split 4 rows/partition over 128 partitions,
row stride 520 (2 left pad + 512 + 6 right pad), 6 row-slots per
partition (slot s = image row 4p+s-1) so every 3x3 tap is a constant
flat offset.

Math (same factorization as v1, ~1.2e-3 vs reference):
  out = num / den    (eps dropped)
  per pair e in {(1,0),(0,1),(1,1),(1,-1)}:
    E_k = DErf(sqrt(S) * (plane_k[+e] - plane_k)), planes (d,c0,c1,c2)
    F_e = E_d * (E_c0 + E_c1 + E_c2)
    FM+ = F*M[+e] (@0 den tap), FM = F*M (@-e den tap)
    Y_c = FM+ * c[+e] (@0 num tap), Z_c = FM * c (@-e num tap)
  Center taps are matmuls on M and a precomputed M*c tile.

Perf structure:
  - odd-column-shifted replicas of inputs built on-chip (ScalarE/DVE)
    so every DVE tensor_tensor op keeps 4B alignment (2x perf mode)
  - er=1 pair fields on 4 rows; the row -1 duplicate comes from
    partition p-1 row 3 via a small SBUF->SBUF DMA
  - fields/products/tap-matmuls interleaved per pair; two row-halves so
    den+3*num PSUM accumulators exactly fill the 8 banks
  - sub+DErf split per plane-pair to pipeline DVE against ScalarE
"""

import math
import numpy as np
import sys

if "/opt/trn_rl_repo" not in sys.path:
    sys.path.insert(0, "/opt/trn_rl_repo")

import concourse.bass as bass
import concourse.tile as tile
from concourse import mybir
from concourse.bass_utils import run_bass_kernel_spmd

# ---- problem constants (hardcoded per spec) ----
B, V, C, H, W = 2, 4, 3, 512, 512
N_CORES = 8
KS = 3
SIG = 0.3 * ((KS - 1) * 0.5 - 1) + 0.8           # 0.8
S = 1.0 / (2.0 * SIG * SIG)                       # 0.78125
SQS = math.sqrt(S)
PHI2 = 4.0 / math.pi

_xs = np.arange(KS, dtype=np.float64)
_gx, _gy = np.meshgrid(_xs, _xs, indexing="xy")
_w = np.exp(-(((_gx - 1) ** 2 + (_gy - 1) ** 2)) * S)
_w = _w / _w.sum()
W0 = float(_w[1, 1])
W1 = float(_w[0, 1])
W2 = float(_w[0, 0])

R = 4                  # data rows per partition
W2C = 520              # row stride (2 left pad + 512 data + 6 right pad)
COL0 = 2
PROD = 4 * W2C         # 2080

# (er, ec, weight-index 0=W1/1=W2); even-column pair first
ES = [(1, 0, 0), (0, 1, 0), (1, 1, 1), (1, -1, 1)]

F16 = mybir.dt.float16
F32 = mybir.dt.float32
AF = mybir.ActivationFunctionType


# ---- walrus single-wait workaround ----------------------------------------
import orjson as _orjson

_SCRATCH = "wsplit_scratch"


def _mk_nop(name, engine, wait):
    return {"name": name, "engine": engine, "ins": [], "outs": [],
            "opcode": "NoOp",
            "sync_info": {"on_wait": [wait], "on_update": []}}


def _split_multiwaits(bir_bytes):
    m = _orjson.loads(bir_bytes)
    for f in m.get("functions", []):
        for bb in f.get("blocks", []):
            out = []
            for ins in bb.get("instructions", []):
                si = ins.get("sync_info")
                waits = (si or {}).get("on_wait") or []
                if len(waits) > 1:
                    for k, w in enumerate(waits[:-1]):
                        nm = f"{ins['name']}-wsplit{k}"
                        out.append(_mk_nop(nm, ins["engine"], w))
                    si["on_wait"] = [waits[-1]]
                out.append(ins)
            bb["instructions"] = out
    return _orjson.dumps(m)


_BUILD_CACHE = {}


def _build_nc():
    nc = bass.Bass()
    xd_in = nc.declare_dram_parameter("xd", [4, 128, 6, W], F16, isOutput=False)
    xm_in = nc.declare_dram_parameter("xm", [128, 6, W], F16, isOutput=False)
    idw_in = nc.declare_dram_parameter("identw", [3, 128, 128], F16, isOutput=False)
    o_out = nc.declare_dram_parameter("out", [C, H, W], F16, isOutput=True)
    nc.dram_tensor(_SCRATCH, [4], F32)

    with tile.TileContext(nc) as tc:
        _emit(nc, tc, xd_in, xm_in, idw_in, o_out)

    orig_to_json = nc.to_json_bytes
    nc.to_json_bytes = lambda: _split_multiwaits(orig_to_json())
    return nc


def _emit(nc, tc, xd_in, xm_in, idw_in, o_out):
    from contextlib import ExitStack
    ctx = ExitStack()
    with ctx:
        persist = ctx.enter_context(tc.tile_pool(name="persist", bufs=1))
        ef_p = ctx.enter_context(tc.tile_pool(name="ef", bufs=2))
        g_p = ctx.enter_context(tc.tile_pool(name="g", bufs=1))
        t_p = ctx.enter_context(tc.tile_pool(name="t", bufs=4))
        ev_p = ctx.enter_context(tc.tile_pool(name="ev", bufs=1))
        psum_p = ctx.enter_context(
            tc.tile_pool(name="psum", bufs=1, space=bass.MemorySpace.PSUM)
        )

        # persistent planes: Dte[even/odd, plane(d,c0..c2), slot 0..5, col]
        Dte = persist.tile([128, 2, 4, 6, W2C], F16, tag="Dte", name="Dte")
        Mte = persist.tile([128, 2, 6, W2C], F16, tag="Mte", name="Mte")
        identw = persist.tile([128, 3, 128], F16, tag="identw", name="identw")
        Ftl = persist.tile([128, 4, 5, W2C], F16, tag="Ftl", name="Ftl")
        mc = persist.tile([128, 3, 4, W2C], F16, tag="mc", name="mc")

        # pad columns + F halo slots (gpsimd: off the DVE critical path)
        nc.gpsimd.memset(Dte[:, 0, :, :, 0:COL0], 0.0)
        nc.gpsimd.memset(Dte[:, 0, :, :, COL0 + W:W2C], 0.0)
        nc.gpsimd.memset(Dte[:, 1, :, :, W2C - 1:W2C], 0.0)
        nc.gpsimd.memset(Mte[:, 0, :, 0:COL0], 0.0)
        nc.gpsimd.memset(Mte[:, 0, :, COL0 + W:W2C], 0.0)
        nc.gpsimd.memset(Mte[:, 1, :, W2C - 1:W2C], 0.0)
        nc.gpsimd.memset(Ftl[:, :, 0:1, :], 0.0)

        # ---- loads: (d,c1,m) on sync, (identw,c0,c2) on scalar so the
        # first sub's plane pair (d,c0) arrives first ----
        nc.scalar.dma_start(identw[:], idw_in.rearrange("j p c -> p j c"))
        nc.sync.dma_start(Dte[:, 0, 0, :, COL0:COL0 + W], xd_in[0])
        nc.scalar.dma_start(Dte[:, 0, 1, :, COL0:COL0 + W], xd_in[1])
        nc.sync.dma_start(Dte[:, 0, 2, :, COL0:COL0 + W], xd_in[2])
        nc.scalar.dma_start(Dte[:, 0, 3, :, COL0:COL0 + W], xd_in[3])
        nc.sync.dma_start(Mte[:, 0, :, COL0:COL0 + W], xm_in[:])
        # (ring order: sync carries d,c1,m; scalar identw,c0,c2)

        Dfe = Dte[:, 0].rearrange("p a b c -> p a (b c)")
        Dfo = Dte[:, 1].rearrange("p a b c -> p a (b c)")
        Mfe = Mte[:, 0].rearrange("p a b -> p (a b)")
        Mfo = Mte[:, 1].rearrange("p a b -> p (a b)")

        def emit_fields(i):
            er, ec, wi = ES[i]
            odd = (ec % 2) != 0
            off = W2C + er * W2C + (ec - 1 if odd else ec)
            src = Dfo if odd else Dfe
            Ez = ef_p.tile([128, 4, PROD], F16, tag="Ez", name="Ez")
            for s in (0, 2):
                nc.vector.tensor_sub(
                    Ez[:, s:s + 2],
                    src[:, s:s + 2, off:off + PROD],
                    Dfe[:, s:s + 2, W2C:W2C + PROD],
                )
                nc.scalar.activation(
                    Ez[:, s:s + 2], Ez[:, s:s + 2], AF.Derivative_Erf,
                    scale=SQS,
                )
            G = g_p.tile([128, PROD], F16, tag="G", name="G")
            nc.vector.tensor_add(G[:], Ez[:, 1], Ez[:, 2])
            nc.vector.tensor_add(G[:], G[:], Ez[:, 3])
            Fout = Ftl[:, i, 1:5, :].rearrange("p a b -> p (a b)")
            nc.vector.tensor_mul(Fout, Ez[:, 0], G[:])
            if er == 1:
                # field halo row -1 from partition p-1's row 3
                nc.sync.dma_start(
                    Ftl[1:128, i, 0:1, :], Ftl[0:127, i, 4:5, :]
                )
            # warm-keepers: paced junk matmuls (overwritten by the real
            # taps' start=True) so the PE HAM stays un-throttled through
            # the field phase
            for r in range(8):
                nc.tensor.matmul(
                    acc0[0][:, 0, :], identw[:, 2],
                    Ftl[:, i, 1 + (r % 4), 0:W],
                    start=True, stop=False, skip_group_check=True,
                )

        def emit_products(i, h0):
            er, ec, wi = ES[i]
            odd = (ec % 2) != 0
            sh = (ec - 1 if odd else ec)
            moff = (h0 + er + 1) * W2C + sh
            Ms = Mfo if odd else Mfe
            Cs = Dfo if odd else Dfe
            fa = Ftl[:, i, h0 + 1:h0 + 3, :]
            fb = Ftl[:, i, h0 + 1 - er:h0 + 3 - er, :]
            FMp = t_p.tile([128, 2, W2C], F16, tag="FMp", name="FMp")
            nc.vector.tensor_mul(
                FMp[:], fa,
                Ms[:, moff:moff + 2 * W2C].rearrange(
                    "p (r c) -> p r c", c=W2C),
            )
            FM = t_p.tile([128, 2, W2C], F16, tag="FM", name="FM")
            nc.vector.tensor_mul(
                FM[:], fb, Mte[:, 0, h0 + 1 - er:h0 + 3 - er, :]
            )
            Y = t_p.tile([128, 3, 2, W2C], F16, tag="Y", name="Y")
            nc.vector.tensor_mul(
                Y[:], FMp.unsqueeze(1).broadcast_to([128, 3, 2, W2C]),
                Cs[:, 1:4, moff:moff + 2 * W2C].rearrange(
                    "p a (r c) -> p a r c", c=W2C),
            )
            Z = t_p.tile([128, 3, 2, W2C], F16, tag="Z", name="Z")
            nc.vector.tensor_mul(
                Z[:], FM.unsqueeze(1).broadcast_to([128, 3, 2, W2C]),
                Dte[:, 0, 1:4, h0 + 1 - er:h0 + 3 - er, :],
            )
            return FMp, FM, Y, Z

        def emit_taps(i, acc, FMp, FM, Y, Z, first, r):
            er, ec, wi = ES[i]
            cb = COL0 - ec
            for pl in range(4):
                a_mv = (FMp[:, r, COL0:COL0 + W] if pl == 0
                        else Y[:, pl - 1, r, COL0:COL0 + W])
                b_mv = (FM[:, r, cb:cb + W] if pl == 0
                        else Z[:, pl - 1, r, cb:cb + W])
                nc.tensor.matmul(
                    acc[pl][:, r, :], identw[:, wi], a_mv,
                    start=first, stop=False,
                )
                nc.tensor.matmul(
                    acc[pl][:, r, :], identw[:, wi], b_mv,
                    start=False, stop=False,
                )

        def emit_center(acc, h0, r):
            nc.tensor.matmul(
                acc[0][:, r, :], identw[:, 2],
                Mte[:, 0, h0 + 1 + r, COL0:COL0 + W],
                start=False, stop=True,
            )
            for ch in range(C):
                nc.tensor.matmul(
                    acc[1 + ch][:, r, :], identw[:, 2],
                    mc[:, ch, h0 + r, COL0:COL0 + W],
                    start=False, stop=True,
                )

        def emit_evac(acc, h0, r):
            # one row at a time so the last chunk's chain is short
            ldn = ev_p.tile([128, W], F32, tag="ldn", name="ldn")
            nc.scalar.activation(ldn[:], acc[0][:, r, :], AF.Ln)
            r16 = ev_p.tile([128, W], F16, tag="r16", name="r16")
            nc.scalar.activation(r16[:], ldn[:], AF.Exp, scale=-1.0)
            for ci in range(C):
                n16 = ev_p.tile([128, W], F16, tag="n16", name="n16")
                nc.scalar.activation(n16[:], acc[1 + ci][:, r, :], AF.Copy)
                o16 = ev_p.tile([128, W], F16, tag="o16", name="o16")
                nc.vector.tensor_mul(o16[:], n16[:], r16[:])
                nc.sync.dma_start(
                    o_out[ci].rearrange("(p r) w -> p r w", r=R)[:, h0 + r, :],
                    o16[:],
                )

        def mk_acc():
            return [
                psum_p.tile([128, 2, W], F32, tag=f"acc{pl}", name=f"acc{pl}")
                for pl in range(4)
            ]

        # ---- interleaved schedule ----
        acc0 = mk_acc()
        emit_fields(0)          # (1,0): even offsets, no replicas needed
        # odd replicas (slots 1..5 suffice) on ScalarE; m*c on DVE
        for k in range(4):
            nc.scalar.activation(
                Dte[:, 1, k, 1:6, 0:W2C - 1], Dte[:, 0, k, 1:6, 1:W2C],
                AF.Copy,
            )
        nc.scalar.activation(
            Mte[:, 1, 1:6, 0:W2C - 1], Mte[:, 0, 1:6, 1:W2C], AF.Copy
        )
        nc.vector.tensor_mul(
            mc[:],
            Mte[:, 0, 1:5, :].unsqueeze(1).broadcast_to([128, 3, 4, W2C]),
            Dte[:, 0, 1:4, 1:5, :],
        )
        emit_fields(1)
        emit_fields(2)
        emit_fields(3)

        # dense product stream, then row-ordered tap streams so row 0's
        # evac overlaps row 1's matmuls
        def half(acc, h0, th):
            for r in range(2):
                for i in range(4):
                    emit_taps(i, acc, *th[i], first=(i == 0), r=r)
                emit_center(acc, h0, r)
                emit_evac(acc, h0, r)

        th0 = [emit_products(i, 0) for i in range(4)]
        half(acc0, 0, th0)
        acc1 = mk_acc()
        th1 = [emit_products(i, 2) for i in range(4)]
        half(acc1, 2, th1)


def _get_nc():
    if "nc" not in _BUILD_CACHE:
        _BUILD_CACHE["nc"] = _build_nc()
    return _BUILD_CACHE["nc"]


def _host_planes(d, c, m):
    """xd [N,4,128,6,512] (d,c0..c2), xm [N,128,6,512] (m); rows
    4p-1..4p+4, fp16, zero halos."""
    from numpy.lib.stride_tricks import as_strided
    N = N_CORES
    stack = np.zeros((N, 5, H + 5, W), np.float16)
    for i in range(N):
        for k, arr in enumerate((d[i], c[i, 0], c[i, 1], c[i, 2], m[i])):
            stack[i, k, 1:H + 1] = arr
    s = stack.strides
    win = as_strided(stack, shape=(N, 5, 128, 6, W),
                     strides=(s[0], s[1], 4 * s[2], s[2], s[3]))
    win = np.ascontiguousarray(win)
    return win[:, 0:4], win[:, 4]


def _run(depth, color, mask, trace=False, **kw):
    nc = _get_nc()
    d = np.asarray(depth, dtype=np.float32).reshape(N_CORES, H, W)
    c = np.asarray(color, dtype=np.float32).reshape(N_CORES, C, H, W)
    m = np.asarray(mask, dtype=np.float32).reshape(N_CORES, H, W)
    xd, xm = _host_planes(d, c, m)
    eye = np.eye(128)
    identw = np.stack(
        [eye * W1, eye * W2, eye * (3.0 * W0 * PHI2)]
    ).astype(np.float16)
    in_maps = [
        {"xd": xd[i], "xm": xm[i], "identw": identw} for i in range(N_CORES)
    ]
    res = run_bass_kernel_spmd(
        nc, in_maps, list(range(N_CORES)), trace=trace, **kw
    )
    out = np.stack([np.asarray(res.results[i]["out"]) for i in range(N_CORES)])
    return out.reshape(B, V, C, H, W).astype(np.float32), res


def kernel(depth, color, mask):
    out, _ = _run(depth, color, mask, trace=False)
    return out


# revision 31
# speedup vs baseline: 1.0196x; 1.0196x over previous
"""Bilateral filter (3x3, sigma=0.8) Trainium2 Bass kernel — v7.

Sharding: fully data-parallel over the fused batch B*V = 8 -> one
(C=3,H=512,W=512) image per NeuronCore, 8 cores.

Per-core layout: H=512 rows split 4 rows/partition over 128 partitions,
row stride 520 (2 left pad + 512 + 6 right pad), 6 row-slots per
partition (slot s = image row 4p+s-1) so every 3x3 tap is a constant
flat offset.

Math (same factorization as v1, ~1.2e-3 vs reference):
  out = num / den    (eps dropped)
  per pair e in {(1,0),(0,1),(1,1),(1,-1)}:
    E_k = DErf(sqrt(S) * (plane_k[+e] - plane_k)), planes (d,c0,c1,c2)
    F_e = E_d * (E_c0 + E_c1 + E_c2)
    FM+ = F*M[+e] (@0 den tap), FM = F*M (@-e den tap)
    Y_c = FM+ * c[+e] (@0 num tap), Z_c = FM * c (@-e num tap)
  Center taps are matmuls on M and a precomputed M*c tile.

Perf structure:
  - odd-column-shifted replicas of inputs built on-chip (ScalarE/DVE)
    so every DVE tensor_tensor op keeps 4B alignment (2x perf mode)
  - er=1 pair fields on 4 rows; the row -1 duplicate comes from
    partition p-1 row 3 via a small SBUF->SBUF DMA
  - fields/products/tap-matmuls interleaved per pair; two row-halves so
    den+3*num PSUM accumulators exactly fill the 8 banks
  - sub+DErf split per plane-pair to pipeline DVE against ScalarE
"""

import math
import numpy as np
import sys

if "/opt/trn_rl_repo" not in sys.path:
    sys.path.insert(0, "/opt/trn_rl_repo")

import concourse.bass as bass
import concourse.tile as tile
from concourse import mybir
from concourse.bass_utils import run_bass_kernel_spmd

# ---- problem constants (hardcoded per spec) ----
B, V, C, H, W = 2, 4, 3, 512, 512
N_CORES = 8
KS = 3
SIG = 0.3 * ((KS - 1) * 0.5 - 1) + 0.8           # 0.8
S = 1.0 / (2.0 * SIG * SIG)                       # 0.78125
SQS = math.sqrt(S)
PHI2 = 4.0 / math.pi

_xs = np.arange(KS, dtype=np.float64)
_gx, _gy = np.meshgrid(_xs, _xs, indexing="xy")
_w = np.exp(-(((_gx - 1) ** 2 + (_gy - 1) ** 2)) * S)
_w = _w / _w.sum()
W0 = float(_w[1, 1])
W1 = float(_w[0, 1])
W2 = float(_w[0, 0])

R = 4                  # data rows per partition
W2C = 520              # row stride (2 left pad + 512 data + 6 right pad)
COL0 = 2
PROD = 4 * W2C         # 2080

# (er, ec, weight-index 0=W1/1=W2); even-column pair first
ES = [(1, 0, 0), (0, 1, 0), (1, 1, 1), (1, -1, 1)]

F16 = mybir.dt.float16
F32 = mybir.dt.float32
AF = mybir.ActivationFunctionType


# ---- walrus single-wait workaround ----------------------------------------
import orjson as _orjson

_SCRATCH = "wsplit_scratch"


def _mk_nop(name, engine, wait):
    return {"name": name, "engine": engine, "ins": [], "outs": [],
            "opcode": "NoOp",
            "sync_info": {"on_wait": [wait], "on_update": []}}


def _split_multiwaits(bir_bytes):
    m = _orjson.loads(bir_bytes)
    for f in m.get("functions", []):
        for bb in f.get("blocks", []):
            out = []
            for ins in bb.get("instructions", []):
                si = ins.get("sync_info")
                waits = (si or {}).get("on_wait") or []
                if len(waits) > 1:
                    for k, w in enumerate(waits[:-1]):
                        nm = f"{ins['name']}-wsplit{k}"
                        out.append(_mk_nop(nm, ins["engine"], w))
                    si["on_wait"] = [waits[-1]]
                out.append(ins)
            bb["instructions"] = out
    return _orjson.dumps(m)


_BUILD_CACHE = {}


def _build_nc():
    nc = bass.Bass()
    xd_in = nc.declare_dram_parameter("xd", [4, 128, 6, W], F16, isOutput=False)
    xm_in = nc.declare_dram_parameter("xm", [128, 6, W], F16, isOutput=False)
    idw_in = nc.declare_dram_parameter("identw", [3, 128, 128], F16, isOutput=False)
    o_out = nc.declare_dram_parameter("out", [C, H, W], F16, isOutput=True)
    nc.dram_tensor(_SCRATCH, [4], F32)

    with tile.TileContext(nc) as tc:
        _emit(nc, tc, xd_in, xm_in, idw_in, o_out)

    orig_to_json = nc.to_json_bytes
    nc.to_json_bytes = lambda: _split_multiwaits(orig_to_json())
    return nc


def _emit(nc, tc, xd_in, xm_in, idw_in, o_out):
    from contextlib import ExitStack
    ctx = ExitStack()
    with ctx:
        persist = ctx.enter_context(tc.tile_pool(name="persist", bufs=1))
        ef_p = ctx.enter_context(tc.tile_pool(name="ef", bufs=2))
        g_p = ctx.enter_context(tc.tile_pool(name="g", bufs=1))
        t_p = ctx.enter_context(tc.tile_pool(name="t", bufs=4))
        ev_p = ctx.enter_context(tc.tile_pool(name="ev", bufs=1))
        psum_p = ctx.enter_context(
            tc.tile_pool(name="psum", bufs=1, space=bass.MemorySpace.PSUM)
        )

        # persistent planes: Dte[even/odd, plane(d,c0..c2), slot 0..5, col]
        Dte = persist.tile([128, 2, 4, 6, W2C], F16, tag="Dte", name="Dte")
        Mte = persist.tile([128, 2, 6, W2C], F16, tag="Mte", name="Mte")
        identw = persist.tile([128, 3, 128], F16, tag="identw", name="identw")
        Ftl = persist.tile([128, 4, 5, W2C], F16, tag="Ftl", name="Ftl")
        mc = persist.tile([128, 3, 4, W2C], F16, tag="mc", name="mc")

        # pad columns + F halo slots (gpsimd: off the DVE critical path)
        nc.gpsimd.memset(Dte[:, 0, :, :, 0:COL0], 0.0)
        nc.gpsimd.memset(Dte[:, 0, :, :, COL0 + W:W2C], 0.0)
        nc.gpsimd.memset(Dte[:, 1, :, :, W2C - 1:W2C], 0.0)
        nc.gpsimd.memset(Mte[:, 0, :, 0:COL0], 0.0)
        nc.gpsimd.memset(Mte[:, 0, :, COL0 + W:W2C], 0.0)
        nc.gpsimd.memset(Mte[:, 1, :, W2C - 1:W2C], 0.0)
        nc.gpsimd.memset(Ftl[:, :, 0:1, :], 0.0)

        # ---- loads: (d,c1,m) on sync, (identw,c0,c2) on scalar so the
        # first sub's plane pair (d,c0) arrives first ----
        nc.scalar.dma_start(identw[:], idw_in.rearrange("j p c -> p j c"))
        nc.sync.dma_start(Dte[:, 0, 0, :, COL0:COL0 + W], xd_in[0])
        nc.scalar.dma_start(Dte[:, 0, 1, :, COL0:COL0 + W], xd_in[1])
        nc.sync.dma_start(Dte[:, 0, 2, :, COL0:COL0 + W], xd_in[2])
        nc.scalar.dma_start(Dte[:, 0, 3, :, COL0:COL0 + W], xd_in[3])
        nc.sync.dma_start(Mte[:, 0, :, COL0:COL0 + W], xm_in[:])
        # (ring order: sync carries d,c1,m; scalar identw,c0,c2)

        Dfe = Dte[:, 0].rearrange("p a b c -> p a (b c)")
        Dfo = Dte[:, 1].rearrange("p a b c -> p a (b c)")
        Mfe = Mte[:, 0].rearrange("p a b -> p (a b)")
        Mfo = Mte[:, 1].rearrange("p a b -> p (a b)")

        def emit_fields(i):
            er, ec, wi = ES[i]
            odd = (ec % 2) != 0
            off = W2C + er * W2C + (ec - 1 if odd else ec)
            src = Dfo if odd else Dfe
            Ez = ef_p.tile([128, 4, PROD], F16, tag="Ez", name="Ez")
            for s in (0, 2):
                nc.vector.tensor_sub(
                    Ez[:, s:s + 2],
                    src[:, s:s + 2, off:off + PROD],
                    Dfe[:, s:s + 2, W2C:W2C + PROD],
                )
                nc.scalar.activation(
                    Ez[:, s:s + 2], Ez[:, s:s + 2], AF.Derivative_Erf,
                    scale=SQS,
                )
            G = g_p.tile([128, PROD], F16, tag="G", name="G")
            nc.vector.tensor_add(G[:], Ez[:, 1], Ez[:, 2])
            nc.vector.tensor_add(G[:], G[:], Ez[:, 3])
            Fout = Ftl[:, i, 1:5, :].rearrange("p a b -> p (a b)")
            nc.vector.tensor_mul(Fout, Ez[:, 0], G[:])
            if er == 1:
                # field halo row -1 from partition p-1's row 3
                nc.sync.dma_start(
                    Ftl[1:128, i, 0:1, :], Ftl[0:127, i, 4:5, :]
                )
            # warm-keepers: paced junk matmuls (overwritten by the real
            # taps' start=True) so the PE HAM stays un-throttled through
            # the field phase
            for r in range(8):
                nc.tensor.matmul(
                    acc0[0][:, 0, :], identw[:, 2],
                    Ftl[:, i, 1 + (r % 4), 0:W],
                    start=True, stop=False, skip_group_check=True,
                )

        def emit_products(i, h0):
            er, ec, wi = ES[i]
            odd = (ec % 2) != 0
            sh = (ec - 1 if odd else ec)
            moff = (h0 + er + 1) * W2C + sh
            Ms = Mfo if odd else Mfe
            Cs = Dfo if odd else Dfe
            fa = Ftl[:, i, h0 + 1:h0 + 3, :]
            fb = Ftl[:, i, h0 + 1 - er:h0 + 3 - er, :]
            FMp = t_p.tile([128, 2, W2C], F16, tag="FMp", name="FMp")
            nc.vector.tensor_mul(
                FMp[:], fa,
                Ms[:, moff:moff + 2 * W2C].rearrange(
                    "p (r c) -> p r c", c=W2C),
            )
            FM = t_p.tile([128, 2, W2C], F16, tag="FM", name="FM")
            nc.vector.tensor_mul(
                FM[:], fb, Mte[:, 0, h0 + 1 - er:h0 + 3 - er, :]
            )
            Y = t_p.tile([128, 3, 2, W2C], F16, tag="Y", name="Y")
            nc.vector.tensor_mul(
                Y[:], FMp.unsqueeze(1).broadcast_to([128, 3, 2, W2C]),
                Cs[:, 1:4, moff:moff + 2 * W2C].rearrange(
                    "p a (r c) -> p a r c", c=W2C),
            )
            Z = t_p.tile([128, 3, 2, W2C], F16, tag="Z", name="Z")
            nc.vector.tensor_mul(
                Z[:], FM.unsqueeze(1).broadcast_to([128, 3, 2, W2C]),
                Dte[:, 0, 1:4, h0 + 1 - er:h0 + 3 - er, :],
            )
            return FMp, FM, Y, Z

        def emit_taps(i, acc, FMp, FM, Y, Z, first, r):
            er, ec, wi = ES[i]
            cb = COL0 - ec
            for pl in range(4):
                a_mv = (FMp[:, r, COL0:COL0 + W] if pl == 0
                        else Y[:, pl - 1, r, COL0:COL0 + W])
                b_mv = (FM[:, r, cb:cb + W] if pl == 0
                        else Z[:, pl - 1, r, cb:cb + W])
                nc.tensor.matmul(
                    acc[pl][:, r, :], identw[:, wi], a_mv,
                    start=first, stop=False,
                )
                nc.tensor.matmul(
                    acc[pl][:, r, :], identw[:, wi], b_mv,
                    start=False, stop=False,
                )

        def emit_center(acc, h0, r):
            nc.tensor.matmul(
                acc[0][:, r, :], identw[:, 2],
                Mte[:, 0, h0 + 1 + r, COL0:COL0 + W],
                start=False, stop=True,
            )
            for ch in range(C):
                nc.tensor.matmul(
                    acc[1 + ch][:, r, :], identw[:, 2],
                    mc[:, ch, h0 + r, COL0:COL0 + W],
                    start=False, stop=True,
                )

        def emit_evac(acc, h0, r):
            # one row at a time so the last chunk's chain is short
            ldn = ev_p.tile([128, W], F32, tag="ldn", name="ldn")
            nc.scalar.activation(ldn[:], acc[0][:, r, :], AF.Ln)
            r16 = ev_p.tile([128, W], F16, tag="r16", name="r16")
            nc.scalar.activation(r16[:], ldn[:], AF.Exp, scale=-1.0)
            for ci in range(C):
                n16 = ev_p.tile([128, W], F16, tag="n16", name="n16")
                nc.scalar.activation(n16[:], acc[1 + ci][:, r, :], AF.Copy)
                o16 = ev_p.tile([128, W], F16, tag="o16", name="o16")
                nc.vector.tensor_mul(o16[:], n16[:], r16[:])
                nc.sync.dma_start(
                    o_out[ci].rearrange("(p r) w -> p r w", r=R)[:, h0 + r, :],
                    o16[:],
                )

        def mk_acc():
            return [
                psum_p.tile([128, 2, W], F32, tag=f"acc{pl}", name=f"acc{pl}")
                for pl in range(4)
            ]

        # ---- interleaved schedule ----
        acc0 = mk_acc()
        emit_fields(0)          # (1,0): even offsets, no replicas needed
        # odd replicas (slots 1..5 suffice), split ScalarE/DVE so neither
        # queue head-of-line-blocks the field chain
        for k in range(4):
            if k % 2 == 0:
                nc.scalar.activation(
                    Dte[:, 1, k, 1:6, 0:W2C - 1], Dte[:, 0, k, 1:6, 1:W2C],
                    AF.Copy,
                )
            else:
                nc.vector.tensor_copy(
                    Dte[:, 1, k, 1:6, 0:W2C - 1], Dte[:, 0, k, 1:6, 1:W2C]
                )
        nc.vector.tensor_copy(
            Mte[:, 1, 1:6, 0:W2C - 1], Mte[:, 0, 1:6, 1:W2C]
        )
        nc.vector.tensor_mul(
            mc[:],
            Mte[:, 0, 1:5, :].unsqueeze(1).broadcast_to([128, 3, 4, W2C]),
            Dte[:, 0, 1:4, 1:5, :],
        )
        emit_fields(1)
        emit_fields(2)
        emit_fields(3)

        # dense product stream, then row-ordered tap streams so row 0's
        # evac overlaps row 1's matmuls
        def half(acc, h0, th):
            for r in range(2):
                for i in range(4):
                    emit_taps(i, acc, *th[i], first=(i == 0), r=r)
                emit_center(acc, h0, r)
                emit_evac(acc, h0, r)

        th0 = [emit_products(i, 0) for i in range(4)]
        half(acc0, 0, th0)
        acc1 = mk_acc()
        th1 = [emit_products(i, 2) for i in range(4)]
        half(acc1, 2, th1)


def _get_nc():
    if "nc" not in _BUILD_CACHE:
        _BUILD_CACHE["nc"] = _build_nc()
    return _BUILD_CACHE["nc"]


def _host_planes(d, c, m):
    """xd [N,4,128,6,512] (d,c0..c2), xm [N,128,6,512] (m); rows
    4p-1..4p+4, fp16, zero halos."""
    from numpy.lib.stride_tricks import as_strided
    N = N_CORES
    stack = np.zeros((N, 5, H + 5, W), np.float16)
    for i in range(N):
        for k, arr in enumerate((d[i], c[i, 0], c[i, 1], c[i, 2], m[i])):
            stack[i, k, 1:H + 1] = arr
    s = stack.strides
    win = as_strided(stack, shape=(N, 5, 128, 6, W),
                     strides=(s[0], s[1], 4 * s[2], s[2], s[3]))
    win = np.ascontiguousarray(win)
    return win[:, 0:4], win[:, 4]


def _run(depth, color, mask, trace=False, **kw):
    nc = _get_nc()
    d = np.asarray(depth, dtype=np.float32).reshape(N_CORES, H, W)
    c = np.asarray(color, dtype=np.float32).reshape(N_CORES, C, H, W)
    m = np.asarray(mask, dtype=np.float32).reshape(N_CORES, H, W)
    xd, xm = _host_planes(d, c, m)
    eye = np.eye(128)
    identw = np.stack(
        [eye * W1, eye * W2, eye * (3.0 * W0 * PHI2)]
    ).astype(np.float16)
    in_maps = [
        {"xd": xd[i], "xm": xm[i], "identw": identw} for i in range(N_CORES)
    ]
    res = run_bass_kernel_spmd(
        nc, in_maps, list(range(N_CORES)), trace=trace, **kw
    )
    out = np.stack([np.asarray(res.results[i]["out"]) for i in range(N_CORES)])
    return out.reshape(B, V, C, H, W).astype(np.float32), res


def kernel(depth, color, mask):
    out, _ = _run(depth, color, mask, trace=False)
    return out


# revision 32
# speedup vs baseline: 1.0579x; 1.0376x over previous
"""Bilateral filter (3x3, sigma=0.8) Trainium2 Bass kernel — v7.

Sharding: fully data-parallel over the fused batch B*V = 8 -> one
(C=3,H=512,W=512) image per NeuronCore, 8 cores.

Per-core layout: H=512 rows split 4 rows/partition over 128 partitions,
row stride 520 (2 left pad + 512 + 6 right pad), 6 row-slots per
partition (slot s = image row 4p+s-1) so every 3x3 tap is a constant
flat offset.

Math (same factorization as v1, ~1.2e-3 vs reference):
  out = num / den    (eps dropped)
  per pair e in {(1,0),(0,1),(1,1),(1,-1)}:
    E_k = DErf(sqrt(S) * (plane_k[+e] - plane_k)), planes (d,c0,c1,c2)
    F_e = E_d * (E_c0 + E_c1 + E_c2)
    FM+ = F*M[+e] (@0 den tap), FM = F*M (@-e den tap)
    Y_c = FM+ * c[+e] (@0 num tap), Z_c = FM * c (@-e num tap)
  Center taps are matmuls on M and a precomputed M*c tile.

Perf structure:
  - odd-column-shifted replicas of inputs built on-chip (ScalarE/DVE)
    so every DVE tensor_tensor op keeps 4B alignment (2x perf mode)
  - er=1 pair fields on 4 rows; the row -1 duplicate comes from
    partition p-1 row 3 via a small SBUF->SBUF DMA
  - fields/products/tap-matmuls interleaved per pair; two row-halves so
    den+3*num PSUM accumulators exactly fill the 8 banks
  - sub+DErf split per plane-pair to pipeline DVE against ScalarE
"""

import math
import numpy as np
import sys

if "/opt/trn_rl_repo" not in sys.path:
    sys.path.insert(0, "/opt/trn_rl_repo")

import concourse.bass as bass
import concourse.tile as tile
from concourse import mybir
from concourse.bass_utils import run_bass_kernel_spmd

# ---- problem constants (hardcoded per spec) ----
B, V, C, H, W = 2, 4, 3, 512, 512
N_CORES = 8
KS = 3
SIG = 0.3 * ((KS - 1) * 0.5 - 1) + 0.8           # 0.8
S = 1.0 / (2.0 * SIG * SIG)                       # 0.78125
SQS = math.sqrt(S)
PHI2 = 4.0 / math.pi

_xs = np.arange(KS, dtype=np.float64)
_gx, _gy = np.meshgrid(_xs, _xs, indexing="xy")
_w = np.exp(-(((_gx - 1) ** 2 + (_gy - 1) ** 2)) * S)
_w = _w / _w.sum()
W0 = float(_w[1, 1])
W1 = float(_w[0, 1])
W2 = float(_w[0, 0])

R = 4                  # data rows per partition
W2C = 520              # row stride (2 left pad + 512 data + 6 right pad)
COL0 = 2
PROD = 4 * W2C         # 2080

# (er, ec, weight-index 0=W1/1=W2); even-column pair first
ES = [(1, 0, 0), (0, 1, 0), (1, 1, 1), (1, -1, 1)]

F16 = mybir.dt.float16
F32 = mybir.dt.float32
AF = mybir.ActivationFunctionType


# ---- walrus single-wait workaround ----------------------------------------
import orjson as _orjson

_SCRATCH = "wsplit_scratch"


def _mk_nop(name, engine, wait):
    return {"name": name, "engine": engine, "ins": [], "outs": [],
            "opcode": "NoOp",
            "sync_info": {"on_wait": [wait], "on_update": []}}


def _split_multiwaits(bir_bytes):
    m = _orjson.loads(bir_bytes)
    for f in m.get("functions", []):
        for bb in f.get("blocks", []):
            out = []
            for ins in bb.get("instructions", []):
                si = ins.get("sync_info")
                waits = (si or {}).get("on_wait") or []
                if len(waits) > 1:
                    for k, w in enumerate(waits[:-1]):
                        nm = f"{ins['name']}-wsplit{k}"
                        out.append(_mk_nop(nm, ins["engine"], w))
                    si["on_wait"] = [waits[-1]]
                out.append(ins)
            bb["instructions"] = out
    return _orjson.dumps(m)


_BUILD_CACHE = {}


def _build_nc():
    nc = bass.Bass()
    xd_in = nc.declare_dram_parameter("xd", [4, 128, 6, W], F16, isOutput=False)
    xm_in = nc.declare_dram_parameter("xm", [128, 6, W], F16, isOutput=False)
    idw_in = nc.declare_dram_parameter("identw", [3, 128, 128], F16, isOutput=False)
    o_out = nc.declare_dram_parameter("out", [C, H, W], F16, isOutput=True)
    nc.dram_tensor(_SCRATCH, [4], F32)

    with tile.TileContext(nc) as tc:
        _emit(nc, tc, xd_in, xm_in, idw_in, o_out)

    orig_to_json = nc.to_json_bytes
    nc.to_json_bytes = lambda: _split_multiwaits(orig_to_json())
    return nc


def _emit(nc, tc, xd_in, xm_in, idw_in, o_out):
    from contextlib import ExitStack
    ctx = ExitStack()
    with ctx:
        persist = ctx.enter_context(tc.tile_pool(name="persist", bufs=1))
        ef_p = ctx.enter_context(tc.tile_pool(name="ef", bufs=2))
        g_p = ctx.enter_context(tc.tile_pool(name="g", bufs=1))
        t_p = ctx.enter_context(tc.tile_pool(name="t", bufs=4))
        ev_p = ctx.enter_context(tc.tile_pool(name="ev", bufs=1))
        psum_p = ctx.enter_context(
            tc.tile_pool(name="psum", bufs=1, space=bass.MemorySpace.PSUM)
        )

        # persistent planes: Dte[even/odd, plane(d,c0..c2), slot 0..5, col]
        Dte = persist.tile([128, 2, 4, 6, W2C], F16, tag="Dte", name="Dte")
        Mte = persist.tile([128, 2, 6, W2C], F16, tag="Mte", name="Mte")
        identw = persist.tile([128, 3, 128], F16, tag="identw", name="identw")
        Ftl = persist.tile([128, 4, 5, W2C], F16, tag="Ftl", name="Ftl")
        mc = persist.tile([128, 3, 4, W2C], F16, tag="mc", name="mc")

        # pad columns + F halo slots (gpsimd: off the DVE critical path)
        nc.gpsimd.memset(Dte[:, 0, :, :, 0:COL0], 0.0)
        nc.gpsimd.memset(Dte[:, 0, :, :, COL0 + W:W2C], 0.0)
        nc.gpsimd.memset(Dte[:, 1, :, :, W2C - 1:W2C], 0.0)
        nc.gpsimd.memset(Mte[:, 0, :, 0:COL0], 0.0)
        nc.gpsimd.memset(Mte[:, 0, :, COL0 + W:W2C], 0.0)
        nc.gpsimd.memset(Mte[:, 1, :, W2C - 1:W2C], 0.0)
        nc.gpsimd.memset(Ftl[:, :, 0:1, :], 0.0)

        # ---- loads: (d,c1,m) on sync, (identw,c0,c2) on scalar so the
        # first sub's plane pair (d,c0) arrives first ----
        nc.scalar.dma_start(identw[:], idw_in.rearrange("j p c -> p j c"))
        nc.sync.dma_start(Dte[:, 0, 0, :, COL0:COL0 + W], xd_in[0])
        nc.scalar.dma_start(Dte[:, 0, 1, :, COL0:COL0 + W], xd_in[1])
        nc.sync.dma_start(Dte[:, 0, 2, :, COL0:COL0 + W], xd_in[2])
        nc.scalar.dma_start(Dte[:, 0, 3, :, COL0:COL0 + W], xd_in[3])
        nc.sync.dma_start(Mte[:, 0, :, COL0:COL0 + W], xm_in[:])
        # (ring order: sync carries d,c1,m; scalar identw,c0,c2)

        Dfe = Dte[:, 0].rearrange("p a b c -> p a (b c)")
        Dfo = Dte[:, 1].rearrange("p a b c -> p a (b c)")
        Mfe = Mte[:, 0].rearrange("p a b -> p (a b)")
        Mfo = Mte[:, 1].rearrange("p a b -> p (a b)")

        def emit_fields(i):
            er, ec, wi = ES[i]
            odd = (ec % 2) != 0
            off = W2C + er * W2C + (ec - 1 if odd else ec)
            src = Dfo if odd else Dfe
            Ez = ef_p.tile([128, 4, PROD], F16, tag="Ez", name="Ez")
            # pair 0 per-plane so its chain starts as each DMA plane lands
            splits = ((0, 1), (1, 2), (2, 3), (3, 4)) if i == 0 else \
                ((0, 2), (2, 4))
            for s0, s1 in splits:
                nc.vector.tensor_sub(
                    Ez[:, s0:s1],
                    src[:, s0:s1, off:off + PROD],
                    Dfe[:, s0:s1, W2C:W2C + PROD],
                )
                nc.scalar.activation(
                    Ez[:, s0:s1], Ez[:, s0:s1], AF.Derivative_Erf,
                    scale=SQS,
                )
            G = g_p.tile([128, PROD], F16, tag="G", name="G")
            nc.vector.tensor_add(G[:], Ez[:, 1], Ez[:, 2])
            nc.vector.tensor_add(G[:], G[:], Ez[:, 3])
            Fout = Ftl[:, i, 1:5, :].rearrange("p a b -> p (a b)")
            nc.vector.tensor_mul(Fout, Ez[:, 0], G[:])
            if er == 1:
                # field halo row -1 from partition p-1's row 3
                nc.sync.dma_start(
                    Ftl[1:128, i, 0:1, :], Ftl[0:127, i, 4:5, :]
                )
            # warm-keepers: paced junk matmuls (overwritten by the real
            # taps' start=True) so the PE HAM stays un-throttled through
            # the field phase
            for r in range(8):
                nc.tensor.matmul(
                    acc0[0][:, 0, :], identw[:, 2],
                    Ftl[:, i, 1 + (r % 4), 0:W],
                    start=True, stop=False, skip_group_check=True,
                )

        def emit_products(i, h0):
            er, ec, wi = ES[i]
            odd = (ec % 2) != 0
            sh = (ec - 1 if odd else ec)
            moff = (h0 + er + 1) * W2C + sh
            Ms = Mfo if odd else Mfe
            Cs = Dfo if odd else Dfe
            fa = Ftl[:, i, h0 + 1:h0 + 3, :]
            fb = Ftl[:, i, h0 + 1 - er:h0 + 3 - er, :]
            FMp = t_p.tile([128, 2, W2C], F16, tag="FMp", name="FMp")
            nc.vector.tensor_mul(
                FMp[:], fa,
                Ms[:, moff:moff + 2 * W2C].rearrange(
                    "p (r c) -> p r c", c=W2C),
            )
            FM = t_p.tile([128, 2, W2C], F16, tag="FM", name="FM")
            nc.vector.tensor_mul(
                FM[:], fb, Mte[:, 0, h0 + 1 - er:h0 + 3 - er, :]
            )
            Y = t_p.tile([128, 3, 2, W2C], F16, tag="Y", name="Y")
            nc.vector.tensor_mul(
                Y[:], FMp.unsqueeze(1).broadcast_to([128, 3, 2, W2C]),
                Cs[:, 1:4, moff:moff + 2 * W2C].rearrange(
                    "p a (r c) -> p a r c", c=W2C),
            )
            Z = t_p.tile([128, 3, 2, W2C], F16, tag="Z", name="Z")
            nc.vector.tensor_mul(
                Z[:], FM.unsqueeze(1).broadcast_to([128, 3, 2, W2C]),
                Dte[:, 0, 1:4, h0 + 1 - er:h0 + 3 - er, :],
            )
            return FMp, FM, Y, Z

        def emit_taps(i, acc, FMp, FM, Y, Z, first, r):
            er, ec, wi = ES[i]
            cb = COL0 - ec
            for pl in range(4):
                a_mv = (FMp[:, r, COL0:COL0 + W] if pl == 0
                        else Y[:, pl - 1, r, COL0:COL0 + W])
                b_mv = (FM[:, r, cb:cb + W] if pl == 0
                        else Z[:, pl - 1, r, cb:cb + W])
                nc.tensor.matmul(
                    acc[pl][:, r, :], identw[:, wi], a_mv,
                    start=first, stop=False,
                )
                nc.tensor.matmul(
                    acc[pl][:, r, :], identw[:, wi], b_mv,
                    start=False, stop=False,
                )

        def emit_center(acc, h0, r):
            nc.tensor.matmul(
                acc[0][:, r, :], identw[:, 2],
                Mte[:, 0, h0 + 1 + r, COL0:COL0 + W],
                start=False, stop=True,
            )
            for ch in range(C):
                nc.tensor.matmul(
                    acc[1 + ch][:, r, :], identw[:, 2],
                    mc[:, ch, h0 + r, COL0:COL0 + W],
                    start=False, stop=True,
                )

        def emit_evac(acc, h0, r):
            # one row at a time so the last chunk's chain is short
            ldn = ev_p.tile([128, W], F32, tag="ldn", name="ldn")
            nc.scalar.activation(ldn[:], acc[0][:, r, :], AF.Ln)
            r16 = ev_p.tile([128, W], F16, tag="r16", name="r16")
            nc.scalar.activation(r16[:], ldn[:], AF.Exp, scale=-1.0)
            for ci in range(C):
                n16 = ev_p.tile([128, W], F16, tag="n16", name="n16")
                nc.scalar.activation(n16[:], acc[1 + ci][:, r, :], AF.Copy)
                o16 = ev_p.tile([128, W], F16, tag="o16", name="o16")
                nc.vector.tensor_mul(o16[:], n16[:], r16[:])
                nc.sync.dma_start(
                    o_out[ci].rearrange("(p r) w -> p r w", r=R)[:, h0 + r, :],
                    o16[:],
                )

        def mk_acc():
            return [
                psum_p.tile([128, 2, W], F32, tag=f"acc{pl}", name=f"acc{pl}")
                for pl in range(4)
            ]

        # ---- interleaved schedule ----
        acc0 = mk_acc()
        emit_fields(0)          # (1,0): even offsets, no replicas needed
        # odd replicas (slots 1..5 suffice), split ScalarE/DVE so neither
        # queue head-of-line-blocks the field chain
        for k in range(4):
            if k % 2 == 0:
                nc.scalar.activation(
                    Dte[:, 1, k, 1:6, 0:W2C - 1], Dte[:, 0, k, 1:6, 1:W2C],
                    AF.Copy,
                )
            else:
                nc.vector.tensor_copy(
                    Dte[:, 1, k, 1:6, 0:W2C - 1], Dte[:, 0, k, 1:6, 1:W2C]
                )
        nc.vector.tensor_copy(
            Mte[:, 1, 1:6, 0:W2C - 1], Mte[:, 0, 1:6, 1:W2C]
        )
        nc.vector.tensor_mul(
            mc[:],
            Mte[:, 0, 1:5, :].unsqueeze(1).broadcast_to([128, 3, 4, W2C]),
            Dte[:, 0, 1:4, 1:5, :],
        )
        emit_fields(1)
        emit_fields(2)
        emit_fields(3)

        # dense product stream, then row-ordered tap streams so row 0's
        # evac overlaps row 1's matmuls
        def half(acc, h0, th):
            for r in range(2):
                for i in range(4):
                    emit_taps(i, acc, *th[i], first=(i == 0), r=r)
                emit_center(acc, h0, r)
                emit_evac(acc, h0, r)

        th0 = [emit_products(i, 0) for i in range(4)]
        half(acc0, 0, th0)
        acc1 = mk_acc()
        th1 = [emit_products(i, 2) for i in range(4)]
        half(acc1, 2, th1)


def _get_nc():
    if "nc" not in _BUILD_CACHE:
        _BUILD_CACHE["nc"] = _build_nc()
    return _BUILD_CACHE["nc"]


def _host_planes(d, c, m):
    """xd [N,4,128,6,512] (d,c0..c2), xm [N,128,6,512] (m); rows
    4p-1..4p+4, fp16, zero halos."""
    from numpy.lib.stride_tricks import as_strided
    N = N_CORES
    stack = np.zeros((N, 5, H + 5, W), np.float16)
    for i in range(N):
        for k, arr in enumerate((d[i], c[i, 0], c[i, 1], c[i, 2], m[i])):
            stack[i, k, 1:H + 1] = arr
    s = stack.strides
    win = as_strided(stack, shape=(N, 5, 128, 6, W),
                     strides=(s[0], s[1], 4 * s[2], s[2], s[3]))
    win = np.ascontiguousarray(win)
    return win[:, 0:4], win[:, 4]


def _run(depth, color, mask, trace=False, **kw):
    nc = _get_nc()
    d = np.asarray(depth, dtype=np.float32).reshape(N_CORES, H, W)
    c = np.asarray(color, dtype=np.float32).reshape(N_CORES, C, H, W)
    m = np.asarray(mask, dtype=np.float32).reshape(N_CORES, H, W)
    xd, xm = _host_planes(d, c, m)
    eye = np.eye(128)
    identw = np.stack(
        [eye * W1, eye * W2, eye * (3.0 * W0 * PHI2)]
    ).astype(np.float16)
    in_maps = [
        {"xd": xd[i], "xm": xm[i], "identw": identw} for i in range(N_CORES)
    ]
    res = run_bass_kernel_spmd(
        nc, in_maps, list(range(N_CORES)), trace=trace, **kw
    )
    out = np.stack([np.asarray(res.results[i]["out"]) for i in range(N_CORES)])
    return out.reshape(B, V, C, H, W).astype(np.float32), res


def kernel(depth, color, mask):
    out, _ = _run(depth, color, mask, trace=False)
    return out
